# revision 1
# baseline (speedup 1.0000x reference)
"""Trainium2 Bass kernel for nn_CausalSelfAttention_61795989455492.

Sharding (8 cores): core c -> batch b = c//2, head-group hg = c%2 (8 of 16
heads). Each core runs QKV projection (its head slice), rotary, sliding-window
attention with joint prefix softmax, and a partial output projection over its
512 channel columns. Host sums the two partials per batch (pair reduce).

Device layout ("transposed attention"):
  - q^T, k^T: (d on partition, t on free) straight out of projection matmuls
  - att^T blocks: (s on partition, t on free); softmax denominator comes from a
    ones-column appended to V (y_aug row 64), so no partition reductions needed
  - window mask folded into PSUM via tiny bf16 identity x mask matmuls
  - exp via ScalarE with scale=1/sqrt(D); no max subtraction (scores are O(5))
"""

import sys
from contextlib import ExitStack

import numpy as np

sys.path.insert(0, "/opt/trn_rl_repo")

import ml_dtypes  # noqa: E402
import concourse.bass as bass  # noqa: E402
import concourse.tile as tile_mod  # noqa: E402
from concourse import bacc  # noqa: E402
from concourse import mybir  # noqa: E402

B, T, C, H, D = 4, 512, 1024, 16, 64
S_PREV, PFX, WINDOW = 1536, 256, 256
ROPE_BASE = 10000.0
MASKVAL = -1.0e5
HPC = 8  # heads per core
NCORES = 8

f32 = mybir.dt.float32
f32r = mybir.dt.float32r
bf16 = mybir.dt.bfloat16

# window geometry per 512-col KV chunk, transposed layout:
# s-block tj -> t-run [T0[tj], T0[tj]+TN[tj])  (t-blocks ti in {tj,tj+1,tj+2})
_T0 = [0, 128, 256, 256]
_TN = [384, 384, 256, 256]
# psum column offset of each tj window inside the (128,1536) chunk tile
_POFF = [0, 512, 1024, 1280]
# exp-output column offset of each tj window inside the (128,1280) tile
_EOFF = [0, 384, 768, 1024]


def _emit(nc, tc, io):
    ctx = ExitStack()
    with ctx:
        const = ctx.enter_context(tc.tile_pool(name="const", bufs=1))
        qkrot = ctx.enter_context(tc.tile_pool(name="qkrot", bufs=1))
        vsb = ctx.enter_context(tc.tile_pool(name="vsb", bufs=1))
        ysb = ctx.enter_context(tc.tile_pool(name="ysb", bufs=1))
        tmp = ctx.enter_context(tc.tile_pool(name="tmp", bufs=3))

        sb_cos = const.tile([128, 512], f32)
        nc.sync.dma_start(out=sb_cos, in_=io["cos2"].ap())
        sb_sin = const.tile([128, 512], f32)
        nc.sync.dma_start(out=sb_sin, in_=io["sin2"].ap())
        sb_I = const.tile([128, 128], bf16)
        nc.sync.dma_start(out=sb_I, in_=io["ident"].ap())
        sb_diag = const.tile([128, 128], bf16)
        nc.sync.dma_start(out=sb_diag, in_=io["diag_tri"].ap())
        sb_bound = const.tile([128, 128], bf16)
        nc.sync.dma_start(out=sb_bound, in_=io["bound_tri"].ap())
        sb_full = const.tile([128, 128], bf16)
        nc.sync.dma_start(out=sb_full, in_=io["full_msk"].ap())
        ones1 = const.tile([1, 64], f32r)
        nc.sync.dma_start(out=ones1, in_=io["ones_row"].ap())
        ones4 = const.tile([128, 4, 1], f32r)
        nc.sync.dma_start(out=ones4, in_=io["ones4"].ap())

        q_rot = [qkrot.tile([128, 512], f32r, name=f"qrot{i}", tag=f"qrot{i}") for i in range(4)]
        k_rot = [qkrot.tile([128, 512], f32r, name=f"krot{i}", tag=f"krot{i}") for i in range(4)]
        v_sb = [vsb.tile([128, 512], f32, name=f"vsb{i}", tag=f"vsb{i}") for i in range(4)]
        y_t = [ysb.tile([128, 512], f32r, name=f"ysb{i}", tag=f"ysb{i}") for i in range(4)]

        # ---------------- phase 1: qkv projection + rotary ----------------
        with tc.tile_pool(name="wqkv", bufs=1) as wpool, \
             tc.tile_pool(name="xt", bufs=1) as xpool, \
             tc.tile_pool(name="projps", bufs=4, space="PSUM") as projps:
            sb_w = []
            for i in range(8):
                w = wpool.tile([128, 1536], f32r, name=f"w{i}", tag=f"w{i}")
                nc.sync.dma_start(out=w, in_=io["w_qkvT"].ap()[i * 128:(i + 1) * 128, :])
                sb_w.append(w)
            sb_x = []
            for i in range(8):
                xt = xpool.tile([128, 512], f32r, name=f"x{i}", tag=f"x{i}")
                nc.sync.dma_start(out=xt, in_=io["xT"].ap()[i * 128:(i + 1) * 128, :])
                sb_x.append(xt)

            # q^T and k^T: m-tiles 0..7 over qkv rows (q: 0..3, k: 4..7)
            for m in range(8):
                ps = projps.tile([128, 512], f32, name="projps", tag="projps")
                for c in range(8):
                    nc.tensor.matmul(
                        ps,
                        lhsT=sb_w[c][:, m * 128:(m + 1) * 128],
                        rhs=sb_x[c],
                        start=(c == 0),
                        stop=(c == 7),
                    )
                # rotary: rot = qk * cos2 + shuffle(qk) * sin2
                rot = q_rot[m] if m < 4 else k_rot[m - 4]
                qsb = tmp.tile([128, 512], f32, name="qsb", tag="qsb")
                nc.vector.tensor_copy(qsb, ps)
                sh = tmp.tile([128, 512], f32, name="sh", tag="sh")
                for dst, src in ((0, 32), (32, 0), (64, 96), (96, 64)):
                    nc.gpsimd.tensor_copy(out=sh[dst:dst + 32, :], in_=qsb[src:src + 32, :])
                nc.vector.tensor_mul(rot, qsb, sb_cos)
                nc.vector.tensor_mul(sh, sh, sb_sin)
                nc.vector.tensor_add(rot, rot, sh)

            # v natural: t-blocks 0..3 -> (t, head*64+d)
            for tb in range(4):
                ps = projps.tile([128, 512], f32, name="projps", tag="projps")
                for c in range(8):
                    nc.tensor.matmul(
                        ps,
                        lhsT=sb_x[c][:, tb * 128:(tb + 1) * 128],
                        rhs=sb_w[c][:, 1024:1536],
                        start=(c == 0),
                        stop=(c == 7),
                    )
                nc.vector.tensor_copy(v_sb[tb], ps)

        # ---------------- phase 2: attention per head ----------------
        with tc.tile_pool(name="kts", bufs=2) as kts_p, \
             tc.tile_pool(name="pref", bufs=2) as pref_p, \
             tc.tile_pool(name="cvn", bufs=2) as cvn_p, \
             tc.tile_pool(name="vaug", bufs=3) as vaug_p, \
             tc.tile_pool(name="expsb", bufs=2) as exp_p, \
             tc.tile_pool(name="exppref", bufs=2) as expp_p, \
             tc.tile_pool(name="rdn", bufs=2) as rdn_p, \
             tc.tile_pool(name="attps", bufs=2, space="PSUM") as attps_p, \
             tc.tile_pool(name="yaug", bufs=2, space="PSUM") as yaug_p:
            kts = None
            for h in range(HPC):
                hrow = (h % 2) * 64
                mt = h // 2
                if h % 2 == 0:
                    kts = kts_p.tile([128, 1536], f32r, name="kts", tag="kts")
                    nc.sync.dma_start(out=kts, in_=io["kT_cache"].ap()[h // 2])
                pref = pref_p.tile([128, 1024], f32, name="pref", tag="pref")
                nc.sync.dma_start(out=pref, in_=io["prefT"].ap()[h])
                cvn = cvn_p.tile([128, 2, 65], f32r, name="cvn", tag="cvn")
                nc.sync.dma_start(
                    out=cvn[:, :, 0:64],
                    in_=io["cache_v_n"].ap()[h].rearrange("(blk p) d -> p blk d", p=128),
                )
                nc.vector.tensor_copy(cvn[:, :, 64:65], ones4[:, 0:2, :])

                yps = yaug_p.tile([128, 512], f32, name="yaug", tag="yaug")

                # prefix: exp then AV (+denominator via ones column)
                expp = expp_p.tile([128, 1024], f32r, name="exppref", tag="exppref")
                nc.scalar.activation(out=expp, in_=pref, func=mybir.ActivationFunctionType.Exp)
                for pb in range(2):
                    nc.tensor.matmul(
                        yps[0:65, :],
                        lhsT=cvn[:, pb, :],
                        rhs=expp[:, pb * 512:(pb + 1) * 512],
                        start=(pb == 0),
                        stop=False,
                        skip_group_check=True,
                    )

                for ck in range(4):
                    aps = attps_p.tile([128, 1536], f32, name="attps", tag="attps")
                    vau = vaug_p.tile([128, 4, 65], f32r, name="vaug", tag="vaug")
                    if ck < 3:
                        nc.sync.dma_start(
                            out=vau[:, :, 0:64],
                            in_=io["v_cache"].ap()[h, ck * 512:(ck + 1) * 512, :]
                            .rearrange("(blk p) d -> p blk d", p=128),
                        )
                    else:
                        for tb in range(4):
                            nc.vector.tensor_copy(
                                vau[:, tb, 0:64], v_sb[tb][:, h * 64:(h + 1) * 64]
                            )
                    nc.vector.tensor_copy(vau[:, :, 64:65], ones4)

                    # QK + mask matmuls per s-block window
                    for tj in range(4):
                        t0, tn, off = _T0[tj], _TN[tj], _POFF[tj]
                        if ck < 3:
                            kblk = kts[hrow:hrow + 64, ck * 512 + tj * 128: ck * 512 + (tj + 1) * 128]
                        else:
                            kblk = k_rot[mt][hrow:hrow + 64, tj * 128:(tj + 1) * 128]
                        qs = q_rot[mt][hrow:hrow + 64, t0:t0 + tn]
                        nc.tensor.matmul(
                            aps[:, off:off + tn],
                            lhsT=kblk,
                            rhs=qs,
                            start=True,
                            stop=False,
                            skip_group_check=True,
                        )
                        dt0 = tj * 128 - t0  # diag block local offset
                        last_mask = (tj >= 2)
                        if tj == 3:  # fully-masked ti=2 block sits at local 0
                            nc.tensor.matmul(
                                aps[:, off:off + 128], lhsT=sb_I, rhs=sb_full,
                                start=False, stop=False, skip_group_check=True,
                            )
                        nc.tensor.matmul(
                            aps[:, off + dt0:off + dt0 + 128], lhsT=sb_I, rhs=sb_diag,
                            start=False, stop=last_mask, skip_group_check=True,
                        )
                        if tj < 2:  # boundary block ti=tj+2 at local 256
                            nc.tensor.matmul(
                                aps[:, off + 256:off + 384], lhsT=sb_I, rhs=sb_bound,
                                start=False, stop=True, skip_group_check=True,
                            )

                    # exp (scale=1/sqrt(D)); two instructions cover the 3 banks
                    ex = exp_p.tile([128, 1280], f32r, name="expsb", tag="expsb")
                    in01 = aps.rearrange("p (w c) -> p w c", c=512)[:, 0:2, 0:384]
                    out01 = ex[:, 0:768].rearrange("p (w c) -> p w c", c=384)
                    nc.scalar.activation(
                        out=out01, in_=in01, func=mybir.ActivationFunctionType.Exp,
                        scale=0.125,
                    )
                    nc.scalar.activation(
                        out=ex[:, 768:1280], in_=aps[:, 1024:1536],
                        func=mybir.ActivationFunctionType.Exp, scale=0.125,
                    )

                    # AV accumulate into y_aug
                    for tj in range(4):
                        t0, tn, eoff = _T0[tj], _TN[tj], _EOFF[tj]
                        nc.tensor.matmul(
                            yps[0:65, t0:t0 + tn],
                            lhsT=vau[:, tj, :],
                            rhs=ex[:, eoff:eoff + tn],
                            start=False,
                            stop=(ck == 3 and tj == 3),
                            skip_group_check=True,
                        )

                # normalize: y^T = y_aug[0:64] * (1/denom) broadcast over d
                rcp = rdn_p.tile([1, 512], f32r, name="rcp", tag="rcp")
                with nc.allow_low_precision(reason="fp32r reciprocal feeds broadcast matmul"):
                    nc.vector.reciprocal(rcp, yps[64:65, :])
                rbp = yaug_p.tile([64, 512], f32, name="rbp", tag="yaug")
                nc.tensor.matmul(rbp, lhsT=ones1, rhs=rcp,
                                 start=True, stop=True)
                rb = rdn_p.tile([64, 512], f32, name="rb", tag="rb")
                nc.vector.tensor_copy(rb, rbp)
                nc.vector.tensor_mul(y_t[mt][hrow:hrow + 64, :], yps[0:64, :], rb)

        # ---------------- phase 3: output projection (partial) ----------------
        with tc.tile_pool(name="wp", bufs=1) as wp_p, \
             tc.tile_pool(name="outsb", bufs=3) as out_p, \
             tc.tile_pool(name="cpps", bufs=3, space="PSUM") as cpps_p:
            wp = []
            for ct in range(4):
                w = wp_p.tile([128, 1024], f32r, name=f"wp{ct}", tag=f"wp{ct}")
                nc.sync.dma_start(out=w, in_=io["w_projT"].ap()[ct * 128:(ct + 1) * 128, :])
                wp.append(w)
            for tb in range(4):
                for ng in range(2):
                    cps = cpps_p.tile([128, 512], f32, name="cpps", tag="cpps")
                    for ct in range(4):
                        nc.tensor.matmul(
                            cps,
                            lhsT=y_t[ct][:, tb * 128:(tb + 1) * 128],
                            rhs=wp[ct][:, ng * 512:(ng + 1) * 512],
                            start=(ct == 0),
                            stop=(ct == 3),
                        )
                    ob = out_p.tile([128, 512], f32, name="outsb", tag="outsb")
                    nc.vector.tensor_copy(ob, cps)
                    nc.sync.dma_start(
                        out=io["out"].ap()[tb * 128:(tb + 1) * 128, ng * 512:(ng + 1) * 512],
                        in_=ob,
                    )


def build_nc():
    nc = bacc.Bacc("TRN2", target_bir_lowering=False, debug=False)
    io = {}
    io["xT"] = nc.declare_dram_parameter("xT", [1024, 512], f32r, isOutput=False)
    io["w_qkvT"] = nc.declare_dram_parameter("w_qkvT", [1024, 1536], f32r, isOutput=False)
    io["kT_cache"] = nc.declare_dram_parameter("kT_cache", [HPC // 2, 128, 1536], f32r, isOutput=False)
    io["v_cache"] = nc.declare_dram_parameter("v_cache", [HPC, 1536, 64], f32r, isOutput=False)
    io["prefT"] = nc.declare_dram_parameter("prefT", [HPC, 128, 1024], f32, isOutput=False)
    io["cache_v_n"] = nc.declare_dram_parameter("cache_v_n", [HPC, 256, 64], f32r, isOutput=False)
    io["w_projT"] = nc.declare_dram_parameter("w_projT", [512, 1024], f32r, isOutput=False)
    io["cos2"] = nc.declare_dram_parameter("cos2", [128, 512], f32, isOutput=False)
    io["sin2"] = nc.declare_dram_parameter("sin2", [128, 512], f32, isOutput=False)
    io["ident"] = nc.declare_dram_parameter("ident", [128, 128], bf16, isOutput=False)
    io["diag_tri"] = nc.declare_dram_parameter("diag_tri", [128, 128], bf16, isOutput=False)
    io["bound_tri"] = nc.declare_dram_parameter("bound_tri", [128, 128], bf16, isOutput=False)
    io["full_msk"] = nc.declare_dram_parameter("full_msk", [128, 128], bf16, isOutput=False)
    io["ones_row"] = nc.declare_dram_parameter("ones_row", [1, 64], f32r, isOutput=False)
    io["ones4"] = nc.declare_dram_parameter("ones4", [128, 4, 1], f32r, isOutput=False)
    io["out"] = nc.declare_dram_parameter("out", [512, 1024], f32, isOutput=True)

    with tile_mod.TileContext(nc) as tc:
        _emit(nc, tc, io)
    nc.finalize()
    return nc


def _rotary_tables(start_index):
    half = D // 2
    inv_freq = 1.0 / (ROPE_BASE ** (np.arange(half, dtype=np.float32) / half))
    pos = (float(start_index) + np.arange(T, dtype=np.float32))
    ang = inv_freq[:, None] * pos[None, :]  # (32, 512): [d, t]
    c = np.cos(ang, dtype=np.float32)
    s = np.sin(ang, dtype=np.float32)
    cos2 = np.tile(c, (4, 1))  # (128, 512)
    sin2 = np.tile(np.concatenate([-s, s], axis=0), (2, 1))  # (128, 512)
    return np.ascontiguousarray(cos2), np.ascontiguousarray(sin2)


def _mask_consts():
    ident = np.eye(128, dtype=ml_dtypes.bfloat16)
    i = np.arange(128)
    diag = np.where(i[:, None] > i[None, :], MASKVAL, 0.0).astype(ml_dtypes.bfloat16)
    bound = np.where(i[None, :] > i[:, None], MASKVAL, 0.0).astype(ml_dtypes.bfloat16)
    full = np.full((128, 128), MASKVAL, dtype=ml_dtypes.bfloat16)
    return ident, diag, bound, full


def make_in_maps(x, c_attn_w, c_proj_w, cached_k, cached_v, att_prefix, cache_v, start_index):
    cos2, sin2 = _rotary_tables(np.asarray(start_index).item())
    ident, diag, bound, full = _mask_consts()
    in_maps = []
    for core in range(NCORES):
        b, hg = core // 2, core % 2
        hs = slice(hg * HPC, (hg + 1) * HPC)
        r0, r1 = hg * 512, (hg + 1) * 512
        wq = c_attn_w[r0:r1]
        wk = c_attn_w[C + r0:C + r1]
        wv = c_attn_w[2 * C + r0:2 * C + r1]
        w_qkvT = np.ascontiguousarray(np.concatenate([wq, wk, wv], axis=0).T)
        p = att_prefix[b, hs].transpose(0, 2, 1)  # (8, 256, 512)
        prefT = np.ascontiguousarray(np.concatenate([p[:, :128], p[:, 128:]], axis=2))
        in_maps.append({
            "xT": np.ascontiguousarray(x[b].T),
            "w_qkvT": w_qkvT,
            "kT_cache": np.ascontiguousarray(
                cached_k[b, hs].transpose(0, 2, 1).reshape(HPC // 2, 128, 1536)),
            "v_cache": np.ascontiguousarray(cached_v[b, hs]),
            "prefT": prefT,
            "cache_v_n": np.ascontiguousarray(cache_v[b, hs]),
            "w_projT": np.ascontiguousarray(c_proj_w[:, r0:r1].T),
            "cos2": cos2,
            "sin2": sin2,
            "ident": ident,
            "ones_row": np.ones((1, 64), np.float32),
            "ones4": np.ones((128, 4, 1), np.float32),
            "diag_tri": diag,
            "bound_tri": bound,
            "full_msk": full,
        })
    return in_maps


_NC_CACHE = {}


def kernel(x, c_attn_w, c_proj_w, cached_k, cached_v, att_prefix, cache_v, start_index):
    x = np.asarray(x, dtype=np.float32)
    c_attn_w = np.asarray(c_attn_w, dtype=np.float32)
    c_proj_w = np.asarray(c_proj_w, dtype=np.float32)
    cached_k = np.asarray(cached_k, dtype=np.float32)
    cached_v = np.asarray(cached_v, dtype=np.float32)
    att_prefix = np.asarray(att_prefix, dtype=np.float32)
    cache_v = np.asarray(cache_v, dtype=np.float32)

    if "nc" not in _NC_CACHE:
        _NC_CACHE["nc"] = build_nc()
    nc = _NC_CACHE["nc"]

    in_maps = make_in_maps(x, c_attn_w, c_proj_w, cached_k, cached_v,
                           att_prefix, cache_v, start_index)
    from concourse.bass_utils import run_bass_kernel_spmd
    res = run_bass_kernel_spmd(nc, in_maps, list(range(NCORES)))
    outs = res.results
    y = np.empty((B, T, C), dtype=np.float32)
    for b in range(B):
        y[b] = outs[2 * b]["out"] + outs[2 * b + 1]["out"]
    return y



# revision 29
# speedup vs baseline: 1.4801x; 1.4801x over previous
"""Trainium2 Bass kernel for nn_CausalSelfAttention_61795989455492.

Sharding (8 cores): core c -> batch b = c//2, head-group hg = c%2 (8 of 16
heads). Each core runs QKV projection (its head slice), rotary, sliding-window
attention with joint prefix softmax, and a partial output projection over its
512 channel columns. Host sums the two partials per batch (pair reduce).

Device layout ("transposed attention"):
  - q^T, k^T: (d on partition, t on free) straight out of projection matmuls;
    d-rows are PERMUTED per head ([0:16,32:48,16:32,48:64]) so the rotary
    half-swap becomes a within-32-partition shuffle (one DVE stream_shuffle).
  - att^T blocks: (s on partition, t on free) in packed slots
    [0:384,384:768,768:1024,1024:1152); softmax denominator comes from a
    ones-column appended to V (y_aug row 64), so no partition reductions.
  - window mask folded into PSUM via batched bf16 identity x mask matmuls
  - exp via ScalarE with scale=1/sqrt(D), bf16 out; no max subtraction
  - most operands bf16 (PE runs f32r at bf16 speed anyway; halves DMA/DVE)
"""

import sys
from contextlib import ExitStack

import numpy as np

sys.path.insert(0, "/opt/trn_rl_repo")

import ml_dtypes  # noqa: E402
import concourse.bass as bass  # noqa: E402
import concourse.tile as tile_mod  # noqa: E402
from concourse import bacc  # noqa: E402
from concourse import mybir  # noqa: E402

B, T, C, H, D = 4, 512, 1024, 16, 64
S_PREV, PFX, WINDOW = 1536, 256, 256
ROPE_BASE = 10000.0
MASKVAL = -1.0e5
HPC = 8  # heads per core
NCORES = 8

f32 = mybir.dt.float32
f32r = mybir.dt.float32r
bf16 = mybir.dt.bfloat16

# window geometry per 512-col KV chunk, transposed layout:
# s-block tj -> t-run [T0[tj], T0[tj]+TN[tj]) at psum offset POFF[tj]
# (bank-aligned so no matmul write crosses a 2KB PSUM bank boundary);
# exp packs slots contiguously into ex at offsets EOFF
_T0 = [0, 128, 256, 384]
_TN = [384, 384, 256, 128]
_POFF = [0, 512, 1024, 1280]
_EOFF = [0, 384, 768, 1024]
_APSW = 1408
_EXPW = 1152

# d-permutation within each head (so rotary swap is within 32 partitions)
_PERM64 = np.concatenate([np.arange(0, 16), np.arange(32, 48),
                          np.arange(16, 32), np.arange(48, 64)])
_SHUF_MASK = list(range(16, 32)) + list(range(0, 16))

# feature toggles (for isolating compiler issues)
USE_BITCAST = False     # f32r-bitcast rhs in the denom broadcast matmul
USE_FAST_RECIP = False  # reciprocal_approx_fast vs vector.reciprocal
USE_SHUFFLE = True      # DVE stream_shuffle vs gpsimd copies for rotary swap
USE_STT = True          # scalar_tensor_tensor for normalize mul


def _emit(nc, tc, io):
    ctx = ExitStack()
    with ctx:
        const = ctx.enter_context(tc.tile_pool(name="const", bufs=1))
        qkrot = ctx.enter_context(tc.tile_pool(name="qkrot", bufs=1))
        vsb = ctx.enter_context(tc.tile_pool(name="vsb", bufs=1))
        ysb = ctx.enter_context(tc.tile_pool(name="ysb", bufs=1))
        kts_p = ctx.enter_context(tc.tile_pool(name="kts", bufs=1))
        pref_p = ctx.enter_context(tc.tile_pool(name="pref", bufs=1))
        vau_p = ctx.enter_context(tc.tile_pool(name="vau", bufs=1))
        wpp = ctx.enter_context(tc.tile_pool(name="wpp", bufs=1))

        # ---- constants ----
        sb_cos = const.tile([128, 512], bf16)
        nc.sync.dma_start(out=sb_cos, in_=io["cos2"].ap())
        sb_sin = const.tile([128, 512], bf16)
        nc.sync.dma_start(out=sb_sin, in_=io["sin2"].ap())
        sb_I = const.tile([128, 128], bf16)
        nc.sync.dma_start(out=sb_I, in_=io["ident"].ap())
        sb_diag = const.tile([128, 128], bf16)
        nc.sync.dma_start(out=sb_diag, in_=io["diag"].ap())
        sb_bound = const.tile([128, 128], bf16)
        nc.sync.dma_start(out=sb_bound, in_=io["bound"].ap())
        ones1 = const.tile([1, 64], f32r if USE_BITCAST else f32)
        nc.sync.dma_start(out=ones1, in_=io["ones_row"].ap())
        ones4 = const.tile([128, 4, 1], bf16)
        nc.sync.dma_start(out=ones4, in_=io["ones4"].ap())

        # ---- bulk input prefetch (front-load every DMA) ----
        sb_x = []
        for i in range(8):
            xt = const.tile([128, 512], bf16, name=f"x{i}", tag=f"x{i}")
            nc.sync.dma_start(out=xt, in_=io["xT"].ap()[i * 128:(i + 1) * 128, :])
            sb_x.append(xt)
        sb_w = []
        for i in range(8):
            w = const.tile([128, 1536], bf16, name=f"w{i}", tag=f"w{i}")
            nc.sync.dma_start(out=w, in_=io["w_qkvT"].ap()[i * 128:(i + 1) * 128, :])
            sb_w.append(w)
        kts = []
        for i in range(HPC // 2):
            kt = kts_p.tile([128, 1536], bf16, name=f"kts{i}", tag=f"kts{i}")
            nc.sync.dma_start(out=kt, in_=io["kT_cache"].ap()[i])
            kts.append(kt)
        vau = []
        pref = []
        for h in range(HPC):
            va = vau_p.tile([128, 18, 65], bf16, name=f"vau{h}", tag=f"vau{h}")
            nc.sync.dma_start(out=va[:, 0:14, :], in_=io["vpack"].ap()[h])
            vau.append(va)
            _DBG["vau"] = vau
            _DBG["kts"] = kts
            pf = pref_p.tile([128, 1024], bf16, name=f"pref{h}", tag=f"pref{h}")
            nc.sync.dma_start(out=pf, in_=io["prefT"].ap()[h])
            pref.append(pf)
        wp = []
        for ct in range(4):
            w = wpp.tile([128, 1024], bf16, name=f"wp{ct}", tag=f"wp{ct}")
            nc.sync.dma_start(out=w, in_=io["w_projT"].ap()[ct * 128:(ct + 1) * 128, :])
            wp.append(w)

        q_rot = [qkrot.tile([128, 512], bf16, name=f"qrot{i}", tag=f"qrot{i}") for i in range(4)]
        k_rot = [qkrot.tile([128, 512], bf16, name=f"krot{i}", tag=f"krot{i}") for i in range(4)]
        v_sb = [vsb.tile([128, 512], bf16, name=f"vsb{i}", tag=f"vsb{i}") for i in range(4)]
        y_t = [ysb.tile([128, 512], bf16, name=f"ysb{i}", tag=f"ysb{i}") for i in range(4)]
        _DBG.update(q_rot=q_rot, k_rot=k_rot, v_sb=v_sb, y_t=y_t)

        # ---------------- phase 1: qkv projection + rotary ----------------
        with tc.tile_pool(name="tmp", bufs=2) as tmp, \
             tc.tile_pool(name="projps", bufs=4, space="PSUM") as projps:
            order = [0, 4, 1, 5, -1, 2, 6, 3, 7, -2, -3, -4]  # qk m-tiles, v tb<0
            for item in order:
                ps = projps.tile([128, 512], f32, name="projps", tag="projps")
                if item >= 0:
                    m = item
                    for c in range(8):
                        nc.tensor.matmul(
                            ps,
                            lhsT=sb_w[c][:, m * 128:(m + 1) * 128],
                            rhs=sb_x[c],
                            start=(c == 0),
                            stop=(c == 7),
                        )
                    # rotary: rot = qk * cos2 + shuffle(qk) * sin2
                    rot = q_rot[m] if m < 4 else k_rot[m - 4]
                    qsb = tmp.tile([128, 512], bf16, name="qsb", tag="qsb")
                    nc.scalar.copy(qsb, ps)
                    sh = tmp.tile([128, 512], bf16, name="sh", tag="sh")
                    if USE_SHUFFLE:
                        nc.vector.stream_shuffle(sh, qsb, _SHUF_MASK)
                    else:
                        for dst in range(0, 128, 32):
                            src = dst ^ 16  # within-32 16-swap under perm
                            for half in (0, 16):
                                nc.gpsimd.tensor_copy(
                                    out=sh[dst + half:dst + half + 16, :],
                                    in_=qsb[(dst + half) ^ 16:((dst + half) ^ 16) + 16, :])
                    nc.vector.tensor_mul(rot, qsb, sb_cos)
                    nc.vector.tensor_mul(sh, sh, sb_sin)
                    nc.vector.tensor_add(rot, rot, sh)
                else:
                    tb = -item - 1
                    for c in range(8):
                        nc.tensor.matmul(
                            ps,
                            lhsT=sb_x[c][:, tb * 128:(tb + 1) * 128],
                            rhs=sb_w[c][:, 1024:1536],
                            start=(c == 0),
                            stop=(c == 7),
                        )
                    nc.scalar.copy(v_sb[tb], ps)

        # ---------------- phase 2: attention per head ----------------
        with tc.tile_pool(name="expsb", bufs=2) as exp_p, \
             tc.tile_pool(name="exppref", bufs=2) as expp_p, \
             tc.tile_pool(name="rcp", bufs=2) as rcp_p, \
             tc.tile_pool(name="attps", bufs=2, space="PSUM") as attps_p, \
             tc.tile_pool(name="yaug", bufs=2, space="PSUM") as yaug_p:
            pending_norm = [None]

            def flush_norm():
                if pending_norm[0] is None:
                    return
                yunn_o, rcp_o, mt_o, hrow_o = pending_norm[0]
                pending_norm[0] = None
                rbp = yaug_p.tile([64, 512], f32, name="rbp", tag="yaug")
                rhs = rcp_o[:, :].bitcast(f32r) if USE_BITCAST else rcp_o[:, :]
                nc.tensor.matmul(rbp, lhsT=ones1, rhs=rhs,
                                 start=True, stop=True)
                # one PSUM operand max per DVE op: yunn is SBUF, rbp is PSUM
                nc.vector.tensor_mul(
                    y_t[mt_o][hrow_o:hrow_o + 64, :], yunn_o, rbp)

            for h in range(HPC):
                hrow = (h % 2) * 64
                mt = h // 2

                # current-chunk V into vau blocks 14:18 (+ ones col)
                for tb in range(4):
                    nc.vector.tensor_copy(
                        vau[h][:, 14 + tb, 0:64], v_sb[tb][:, h * 64:(h + 1) * 64])
                nc.vector.tensor_copy(vau[h][:, 14:18, 64:65], ones4)

                # prefix exp
                expp = expp_p.tile([128, 1024], bf16, name="exppref", tag="exppref")
                nc.scalar.activation(out=expp, in_=pref[h],
                                     func=mybir.ActivationFunctionType.Exp)

                yps = None  # allocated after previous head's flush_norm
                exs = [None] * 4

                for ck in range(4):
                    aps = attps_p.tile([128, _APSW], f32, name="attps", tag="attps")
                    # QK matmuls per s-block window; masks (diag triangle at
                    # slot start, bound triangle at +256 for slots 0,1) must
                    # follow their QK immediately: a later start=True in the
                    # same 2KB psum bank re-arms the bank's pending-zero, and
                    # a mask landing on a pending region would OVERWRITE the
                    # scores instead of accumulating.
                    for tj in range(4):
                        t0, tn, off = _T0[tj], _TN[tj], _POFF[tj]
                        if ck < 3:
                            kblk = kts[mt][hrow:hrow + 64,
                                           ck * 512 + tj * 128: ck * 512 + (tj + 1) * 128]
                        else:
                            kblk = k_rot[mt][hrow:hrow + 64, tj * 128:(tj + 1) * 128]
                        nc.tensor.matmul(
                            aps[:, off:off + tn],
                            lhsT=kblk,
                            rhs=q_rot[mt][hrow:hrow + 64, t0:t0 + tn],
                            start=True,
                            stop=False,
                            skip_group_check=True,
                        )
                        nc.tensor.matmul(
                            aps[:, off:off + 128], lhsT=sb_I, rhs=sb_diag,
                            start=False, stop=(tj == 3), skip_group_check=True,
                        )
                        if tj < 2:
                            nc.tensor.matmul(
                                aps[:, off + 256:off + 384], lhsT=sb_I, rhs=sb_bound,
                                start=False, stop=False, skip_group_check=True,
                            )

                    # exp (scale=1/sqrt(D)), bf16 out, packed into ex
                    ex = exp_p.tile([128, _EXPW], bf16, name="expsb", tag="expsb")
                    nc.scalar.activation(
                        out=ex[:, 0:768].rearrange("p (w c) -> p w c", c=384),
                        in_=aps[:, 0:1024].rearrange("p (w c) -> p w c", c=512)[:, :, 0:384],
                        func=mybir.ActivationFunctionType.Exp, scale=0.125,
                    )
                    nc.scalar.activation(
                        out=ex[:, 768:1152], in_=aps[:, 1024:1408],
                        func=mybir.ActivationFunctionType.Exp, scale=0.125,
                    )
                    exs[ck] = ex
                    _DBG.setdefault("ex_all", []).append(ex)

                    # software pipeline: deferred work interleaves with QK
                    if ck == 0:
                        flush_norm()  # previous head's normalize
                    elif ck == 1:
                        yps = yaug_p.tile([128, 512], f32, name="yaug", tag="yaug")
                        # prefix AV (+denominator via ones column)
                        for pb in range(2):
                            nc.tensor.matmul(
                                yps[0:65, :],
                                lhsT=vau[h][:, pb, :],
                                rhs=expp[:, pb * 512:(pb + 1) * 512],
                                start=(pb == 0),
                                stop=False,
                                skip_group_check=True,
                            )
                        self_av(nc, yps, vau[h], exs[0], 0)
                    else:
                        self_av(nc, yps, vau[h], exs[ck - 1], ck - 1)
                self_av(nc, yps, vau[h], exs[3], 3, last=True)

                # 1/denominator (row 64), unnormalized y to SBUF (frees psum),
                # deferred broadcast+mul
                rcp1 = rcp_p.tile([1, 512], f32, name="rcp", tag="rcp")
                if USE_FAST_RECIP:
                    nc.vector.reciprocal_approx_fast(out=rcp1, in_=yps[64:65, :])
                else:
                    with nc.allow_low_precision(reason="recip feeds bcast matmul"):
                        nc.vector.reciprocal(rcp1, yps[64:65, :])
                yunn = rcp_p.tile([64, 512], bf16, name="yunn", tag="yunn")
                nc.vector.tensor_copy(yunn, yps[0:64, :])
                pending_norm[0] = (yunn, rcp1, mt, hrow)
            flush_norm()

        # ---------------- phase 3: output projection (partial) ----------------
        with tc.tile_pool(name="outsb", bufs=3) as out_p, \
             tc.tile_pool(name="cpps", bufs=3, space="PSUM") as cpps_p:
            for tb in range(4):
                for ng in range(2):
                    cps = cpps_p.tile([128, 512], f32, name="cpps", tag="cpps")
                    for ct in range(4):
                        nc.tensor.matmul(
                            cps,
                            lhsT=y_t[ct][:, tb * 128:(tb + 1) * 128],
                            rhs=wp[ct][:, ng * 512:(ng + 1) * 512],
                            start=(ct == 0),
                            stop=(ct == 3),
                        )
                    ob = out_p.tile([128, 512], f32, name="outsb", tag="outsb")
                    nc.scalar.copy(ob, cps)
                    nc.sync.dma_start(
                        out=io["out"].ap()[tb * 128:(tb + 1) * 128, ng * 512:(ng + 1) * 512],
                        in_=ob,
                    )


def self_av(nc, yps, va, ex, ck, last=False):
    # AV accumulate into y_aug; vau blocks 2..13 cached chunks, 14..17 current
    for tj in range(4):
        t0, tn, eoff = _T0[tj], _TN[tj], _EOFF[tj]
        blk = (2 + ck * 4 + tj) if ck < 3 else (14 + tj)
        nc.tensor.matmul(
            yps[0:65, t0:t0 + tn],
            lhsT=va[:, blk, :],
            rhs=ex[:, eoff:eoff + tn],
            start=False,
            stop=(last and tj == 3),
            skip_group_check=True,
        )


def build_nc():
    nc = bacc.Bacc("TRN2", target_bir_lowering=False, debug=False)
    io = {}
    io["xT"] = nc.declare_dram_parameter("xT", [1024, 512], bf16, isOutput=False)
    io["w_qkvT"] = nc.declare_dram_parameter("w_qkvT", [1024, 1536], bf16, isOutput=False)
    io["kT_cache"] = nc.declare_dram_parameter("kT_cache", [HPC // 2, 128, 1536], bf16, isOutput=False)
    io["vpack"] = nc.declare_dram_parameter("vpack", [HPC, 128, 14, 65], bf16, isOutput=False)
    io["prefT"] = nc.declare_dram_parameter("prefT", [HPC, 128, 1024], bf16, isOutput=False)
    io["w_projT"] = nc.declare_dram_parameter("w_projT", [512, 1024], bf16, isOutput=False)
    io["cos2"] = nc.declare_dram_parameter("cos2", [128, 512], bf16, isOutput=False)
    io["sin2"] = nc.declare_dram_parameter("sin2", [128, 512], bf16, isOutput=False)
    io["ident"] = nc.declare_dram_parameter("ident", [128, 128], bf16, isOutput=False)
    io["diag"] = nc.declare_dram_parameter("diag", [128, 128], bf16, isOutput=False)
    io["bound"] = nc.declare_dram_parameter("bound", [128, 128], bf16, isOutput=False)
    io["ones_row"] = nc.declare_dram_parameter("ones_row", [1, 64],
                                               f32r if USE_BITCAST else f32,
                                               isOutput=False)
    io["ones4"] = nc.declare_dram_parameter("ones4", [128, 4, 1], bf16, isOutput=False)
    io["out"] = nc.declare_dram_parameter("out", [512, 1024], f32, isOutput=True)

    with tile_mod.TileContext(nc) as tc:
        _emit(nc, tc, io)
    nc.finalize()
    return nc


def _rotary_tables(start_index):
    half = D // 2
    inv_freq = 1.0 / (ROPE_BASE ** (np.arange(half, dtype=np.float32) / half))
    pos = (float(start_index) + np.arange(T, dtype=np.float32))
    ang = inv_freq[:, None] * pos[None, :]  # (32, 512): [d, t]
    c = np.cos(ang, dtype=np.float32)
    s = np.sin(ang, dtype=np.float32)
    cos2 = np.tile(c, (4, 1))  # (128, 512)
    sin2 = np.tile(np.concatenate([-s, s], axis=0), (2, 1))  # (128, 512)
    perm128 = np.concatenate([_PERM64, 64 + _PERM64])
    return (np.ascontiguousarray(cos2[perm128]).astype(ml_dtypes.bfloat16),
            np.ascontiguousarray(sin2[perm128]).astype(ml_dtypes.bfloat16))


def _mask_consts():
    ident = np.eye(128, dtype=ml_dtypes.bfloat16)
    i = np.arange(128)
    diag = np.where(i[:, None] > i[None, :], MASKVAL, 0.0).astype(ml_dtypes.bfloat16)
    bound = np.where(i[None, :] > i[:, None], MASKVAL, 0.0).astype(ml_dtypes.bfloat16)
    return ident, diag, bound


def make_in_maps(x, c_attn_w, c_proj_w, cached_k, cached_v, att_prefix, cache_v, start_index):
    cos2, sin2 = _rotary_tables(np.asarray(start_index).item())
    ident, diag, bound = _mask_consts()
    qk_perm = np.concatenate([64 * h + _PERM64 for h in range(HPC)])
    bfc = ml_dtypes.bfloat16
    in_maps = []
    for core in range(NCORES):
        b, hg = core // 2, core % 2
        hs = slice(hg * HPC, (hg + 1) * HPC)
        r0, r1 = hg * 512, (hg + 1) * 512
        wq = c_attn_w[r0:r1][qk_perm]
        wk = c_attn_w[C + r0:C + r1][qk_perm]
        wv = c_attn_w[2 * C + r0:2 * C + r1]
        w_qkvT = np.ascontiguousarray(
            np.concatenate([wq, wk, wv], axis=0).T).astype(bfc)
        p = att_prefix[b, hs].transpose(0, 2, 1)  # (8, 256, 512)
        prefT = np.ascontiguousarray(
            np.concatenate([p[:, :128], p[:, 128:]], axis=2)).astype(bfc)
        kb = cached_k[b, hs][:, :, _PERM64]  # (8, 1536, 64) perm d
        kT_cache = np.ascontiguousarray(
            kb.transpose(0, 2, 1).reshape(HPC // 2, 128, 1536)).astype(bfc)
        # vpack: per head [prefix V (2 blk) | cached V (12 blk)] + ones col
        vp = np.concatenate([
            cache_v[b, hs].reshape(HPC, 2, 128, D),
            cached_v[b, hs].reshape(HPC, 12, 128, D),
        ], axis=1)  # (8, 14, 128, 64)
        vpack = np.empty((HPC, 128, 14, 65), dtype=bfc)
        vpack[:, :, :, 0:64] = vp.transpose(0, 2, 1, 3).astype(bfc)
        vpack[:, :, :, 64] = 1.0
        in_maps.append({
            "xT": np.ascontiguousarray(x[b].T).astype(bfc),
            "w_qkvT": w_qkvT,
            "kT_cache": kT_cache,
            "vpack": np.ascontiguousarray(vpack),
            "prefT": prefT,
            "w_projT": np.ascontiguousarray(c_proj_w[:, r0:r1].T).astype(bfc),
            "cos2": cos2,
            "sin2": sin2,
            "ident": ident,
            "diag": diag,
            "bound": bound,
            "ones_row": np.ones((1, 64), np.float32),
            "ones4": np.ones((128, 4, 1), ml_dtypes.bfloat16),
        })
    return in_maps


_NC_CACHE = {}
_DBG = {}


def kernel(x, c_attn_w, c_proj_w, cached_k, cached_v, att_prefix, cache_v, start_index):
    x = np.asarray(x, dtype=np.float32)
    c_attn_w = np.asarray(c_attn_w, dtype=np.float32)
    c_proj_w = np.asarray(c_proj_w, dtype=np.float32)
    cached_k = np.asarray(cached_k, dtype=np.float32)
    cached_v = np.asarray(cached_v, dtype=np.float32)
    att_prefix = np.asarray(att_prefix, dtype=np.float32)
    cache_v = np.asarray(cache_v, dtype=np.float32)

    if "nc" not in _NC_CACHE:
        _NC_CACHE["nc"] = build_nc()
    nc = _NC_CACHE["nc"]

    in_maps = make_in_maps(x, c_attn_w, c_proj_w, cached_k, cached_v,
                           att_prefix, cache_v, start_index)
    from concourse.bass_utils import run_bass_kernel_spmd
    res = run_bass_kernel_spmd(nc, in_maps, list(range(NCORES)))
    outs = res.results
    y = np.empty((B, T, C), dtype=np.float32)
    for b in range(B):
        y[b] = outs[2 * b]["out"] + outs[2 * b + 1]["out"]
    return y


# revision 38
# speedup vs baseline: 1.5863x; 1.0717x over previous
"""Trainium2 Bass kernel for nn_CausalSelfAttention_61795989455492.

Sharding (8 cores): core c -> batch b = c//2, head-group hg = c%2 (8 of 16
heads). Each core runs QKV projection (its head slice), rotary, sliding-window
attention with joint prefix softmax, and a partial output projection over its
512 channel columns. Host sums the two partials per batch (pair reduce).

Device layout ("transposed attention"):
  - q^T, k^T: (d on partition, t on free) straight out of projection matmuls;
    d-rows are PERMUTED per head ([0:16,32:48,16:32,48:64]) so the rotary
    half-swap becomes a within-32-partition DVE stream_shuffle.
  - att^T blocks: (s on partition, t on free); psum slots bank-packed as
    bank0=[slot0(384)|slot3(128)] bank1=[slot1(384)] bank2=[slot2(256)] so
    diag/bound mask matmuls batch two-at-a-time within a bank.
  - softmax denominator via ones-column appended to V (y_aug row 65);
    reciprocal = exp(-ln(denom)) on ScalarE; broadcast across the 64 d-rows
    via a partition-broadcast SBUF->SBUF DMA (no PE involvement).
  - exp via ScalarE with scale=1/sqrt(D), bf16 out; no max subtraction
  - bf16 operands everywhere (PE runs f32r at bf16 speed; halves DMA/DVE)
  - attention processes TWO heads (one kts pair) interleaved to keep the PE
    dense enough that the HAM clock-gate holds 2.4 GHz.
  - input DMA dispatches split across the two HWDGE queues (sync + scalar).
"""

import sys
from contextlib import ExitStack

import numpy as np

sys.path.insert(0, "/opt/trn_rl_repo")

import ml_dtypes  # noqa: E402
import concourse.bass as bass  # noqa: E402
import concourse.tile as tile_mod  # noqa: E402
from concourse import bacc  # noqa: E402
from concourse import mybir  # noqa: E402

B, T, C, H, D = 4, 512, 1024, 16, 64
S_PREV, PFX, WINDOW = 1536, 256, 256
ROPE_BASE = 10000.0
MASKVAL = -1.0e5
HPC = 8  # heads per core
NCORES = 8

f32 = mybir.dt.float32
f32r = mybir.dt.float32r
bf16 = mybir.dt.bfloat16

# window geometry per 512-col KV chunk, transposed layout:
# s-block tj -> t-run [T0[tj], T0[tj]+TN[tj]) at psum offset POFF[tj]
# (bank-packed: no matmul write crosses a 2KB PSUM bank; diag masks sit at
# +0 and bound masks at +256 of their slot so pairs batch within a bank)
# exp packs slots contiguously into ex at offsets EOFF
_T0 = [0, 128, 256, 384]
_TN = [384, 384, 256, 128]
_POFF = [0, 512, 1024, 384]
_EOFF = [0, 512, 896, 384]
_APSW = 1280
_EXPW = 1152

# d-permutation within each head (so rotary swap is within 32 partitions)
_PERM64 = np.concatenate([np.arange(0, 16), np.arange(32, 48),
                          np.arange(16, 32), np.arange(48, 64)])
_SHUF_MASK = list(range(16, 32)) + list(range(0, 16))

# constpack column layout (bf16): cos2 | sin2 | ident | diag | combo | ones4
# combo holds diag at +0 and bound at +256 (gap keeps the [128,2,128] mask AP
# non-contiguous so it isn't flattened and matches the strided psum out)
_CP_COS, _CP_SIN, _CP_ID, _CP_DG, _CP_CB, _CP_O4 = 0, 512, 1024, 1152, 1280, 1792
_CPW = 1796

USE_DMA_BCAST = True  # partition-broadcast SBUF->SBUF DMA for 1/denom


def _emit(nc, tc, io):
    ctx = ExitStack()
    with ctx:
        const = ctx.enter_context(tc.tile_pool(name="const", bufs=1))
        qkrot = ctx.enter_context(tc.tile_pool(name="qkrot", bufs=1))
        vsb = ctx.enter_context(tc.tile_pool(name="vsb", bufs=1))
        ysb = ctx.enter_context(tc.tile_pool(name="ysb", bufs=1))
        vau_p = ctx.enter_context(tc.tile_pool(name="vau", bufs=1))
        pref_p = ctx.enter_context(tc.tile_pool(name="pref", bufs=1))

        # ---- input DMA: phase-1 critical tensors on the sync queue ----
        x_t = const.tile([128, 8, 512], bf16, name="xt", tag="xt")
        nc.sync.dma_start(out=x_t, in_=io["xT"].ap())
        w_t = []
        for s, nm in enumerate(("wq", "wk", "wv")):
            w = const.tile([128, 8, 512], bf16, name=nm, tag=nm)
            nc.sync.dma_start(out=w, in_=io[nm].ap())
            w_t.append(w)
        cp = const.tile([128, _CPW], bf16, name="constpack", tag="constpack")
        nc.sync.dma_start(out=cp, in_=io["constpack"].ap())

        # ---- attention-side tensors on the scalar HWDGE queue ----
        kts = const.tile([128, 4, 1536], bf16, name="kts", tag="kts")
        nc.scalar.dma_start(out=kts, in_=io["kT_cache"].ap())
        vau = []
        pref = []
        for h in range(HPC):
            va = vau_p.tile([128, 18, 65], bf16, name=f"vau{h}", tag=f"vau{h}")
            vau.append(va)
            pf = pref_p.tile([128, 1024], bf16, name=f"pref{h}", tag=f"pref{h}")
            pref.append(pf)
        for h in range(2):  # first pair up-front
            nc.scalar.dma_start(out=vau[h][:, 0:14, :], in_=io["vpack"].ap()[h])
            nc.scalar.dma_start(out=pref[h], in_=io["prefT"].ap()[h])

        sb_cos = cp[:, _CP_COS:_CP_COS + 512]
        sb_sin = cp[:, _CP_SIN:_CP_SIN + 512]
        sb_I = cp[:, _CP_ID:_CP_ID + 128]
        sb_diag = cp[:, _CP_DG:_CP_DG + 128]
        sb_bound = cp[:, _CP_CB + 256:_CP_CB + 384]
        ones4 = cp[:, _CP_O4:_CP_O4 + 4].rearrange("p (a b) -> p a b", b=1)

        q_rot = [qkrot.tile([128, 512], bf16, name=f"qrot{i}", tag=f"qrot{i}") for i in range(4)]
        k_rot = [qkrot.tile([128, 512], bf16, name=f"krot{i}", tag=f"krot{i}") for i in range(4)]
        v_sb = [vsb.tile([128, 512], bf16, name=f"vsb{i}", tag=f"vsb{i}") for i in range(4)]
        y_t = [ysb.tile([128, 512], bf16, name=f"ysb{i}", tag=f"ysb{i}") for i in range(4)]
        _DBG.update(q_rot=q_rot, k_rot=k_rot, v_sb=v_sb, y_t=y_t, vau=vau)

        # ---------------- phase 1: qkv projection + rotary ----------------
        with tc.tile_pool(name="tmp", bufs=2) as tmp, \
             tc.tile_pool(name="projps", bufs=4, space="PSUM") as projps:
            # (section, mtile); v-items are ('v', tb)
            order = [(0, 0), (1, 0), (0, 1), (1, 1), (2, 0), (0, 2), (1, 2),
                     (2, 1), (0, 3), (1, 3), (2, 2), (2, 3)]
            for sect, m in order:
                ps = projps.tile([128, 512], f32, name="projps", tag="projps")
                if sect < 2:
                    for c in range(8):
                        nc.tensor.matmul(
                            ps,
                            lhsT=w_t[sect][:, c, m * 128:(m + 1) * 128],
                            rhs=x_t[:, c, :],
                            start=(c == 0),
                            stop=(c == 7),
                        )
                    # rotary: rot = qk * cos2 + shuffle(qk) * sin2
                    rot = q_rot[m] if sect == 0 else k_rot[m]
                    qsb = tmp.tile([128, 512], bf16, name="qsb", tag="qsb")
                    nc.scalar.copy(qsb, ps)
                    sh = tmp.tile([128, 512], bf16, name="sh", tag="sh")
                    nc.vector.stream_shuffle(sh, qsb, _SHUF_MASK)
                    nc.vector.tensor_mul(rot, qsb, sb_cos)
                    nc.vector.tensor_mul(sh, sh, sb_sin)
                    nc.vector.tensor_add(rot, rot, sh)
                else:
                    tb = m
                    for c in range(8):
                        nc.tensor.matmul(
                            ps,
                            lhsT=x_t[:, c, tb * 128:(tb + 1) * 128],
                            rhs=w_t[2][:, c, :],
                            start=(c == 0),
                            stop=(c == 7),
                        )
                    nc.vector.tensor_copy(v_sb[tb], ps)

        # remaining attention inputs (scalar HWDGE queue, after phase-1 work)
        for h in range(2, HPC):
            nc.scalar.dma_start(out=vau[h][:, 0:14, :], in_=io["vpack"].ap()[h])
            nc.scalar.dma_start(out=pref[h], in_=io["prefT"].ap()[h])
        wp = const.tile([128, 4, 1024], bf16, name="wp", tag="wp")
        nc.scalar.dma_start(out=wp, in_=io["w_projT"].ap())

        # ---------------- phase 2: attention, two heads interleaved ----------------
        with tc.tile_pool(name="expsb", bufs=4) as exp_p, \
             tc.tile_pool(name="exppref", bufs=2) as expp_p, \
             tc.tile_pool(name="rcp", bufs=4) as rcp_p, \
             tc.tile_pool(name="attps", bufs=2, space="PSUM") as attps_p, \
             tc.tile_pool(name="yaug", bufs=2, space="PSUM") as yaug_p:
            pending = []

            def qk_masks(h, ck):
                hrow = (h % 2) * 64
                mt = h // 2
                aps = attps_p.tile([128, _APSW], f32, name="attps", tag="attps")

                def qk(tj):
                    t0, tn, off = _T0[tj], _TN[tj], _POFF[tj]
                    if ck < 3:
                        kblk = kts[hrow:hrow + 64, mt,
                                   ck * 512 + tj * 128: ck * 512 + (tj + 1) * 128]
                    else:
                        kblk = k_rot[mt][hrow:hrow + 64, tj * 128:(tj + 1) * 128]
                    nc.tensor.matmul(
                        aps[:, off:off + tn],
                        lhsT=kblk,
                        rhs=q_rot[mt][hrow:hrow + 64, t0:t0 + tn],
                        start=True, stop=False, skip_group_check=True,
                    )

                # bank0: slot0 then its masks, then slot3 (whose start=True
                # re-arms bank0's pending-zero) then its diag
                qk(0)
                nc.tensor.matmul(
                    aps[:, 0:128], lhsT=sb_I, rhs=sb_diag,
                    start=False, stop=False, skip_group_check=True,
                )
                nc.tensor.matmul(
                    aps[:, 256:384], lhsT=sb_I, rhs=sb_bound,
                    start=False, stop=False, skip_group_check=True,
                )
                qk(3)
                nc.tensor.matmul(
                    aps[:, 384:512], lhsT=sb_I, rhs=sb_diag,
                    start=False, stop=False, skip_group_check=True,
                )
                qk(1)
                nc.tensor.matmul(
                    aps[:, 512:640], lhsT=sb_I, rhs=sb_diag,
                    start=False, stop=False, skip_group_check=True,
                )
                nc.tensor.matmul(
                    aps[:, 768:896], lhsT=sb_I, rhs=sb_bound,
                    start=False, stop=False, skip_group_check=True,
                )
                qk(2)
                nc.tensor.matmul(
                    aps[:, 1024:1152], lhsT=sb_I, rhs=sb_diag,
                    start=False, stop=True, skip_group_check=True,
                )
                # exp (scale=1/sqrt(D)), bf16 out, slots packed into ex
                ex = exp_p.tile([128, _EXPW], bf16, name="expsb", tag="expsb")
                nc.scalar.activation(
                    out=ex[:, 0:896], in_=aps[:, 0:896],
                    func=mybir.ActivationFunctionType.Exp, scale=0.125,
                )
                nc.scalar.activation(
                    out=ex[:, 896:1152], in_=aps[:, 1024:1280],
                    func=mybir.ActivationFunctionType.Exp, scale=0.125,
                )
                return ex

            def av(yps, h, ex, ck, last):
                for tj in range(4):
                    t0, tn, eoff = _T0[tj], _TN[tj], _EOFF[tj]
                    blk = (2 + ck * 4 + tj) if ck < 3 else (14 + tj)
                    nc.tensor.matmul(
                        yps[0:65, t0:t0 + tn],
                        lhsT=vau[h][:, blk, :],
                        rhs=ex[:, eoff:eoff + tn],
                        start=False,
                        stop=(last and tj == 3),
                        skip_group_check=True,
                    )

            def flush_pending():
                while pending:
                    rb_o, yunn_o, mt_o, hrow_o = pending.pop(0)
                    nc.vector.tensor_mul(
                        y_t[mt_o][hrow_o:hrow_o + 64, :], yunn_o, rb_o)

            for p in range(4):
                A, B = 2 * p, 2 * p + 1
                exs = {}
                yp = {}
                for h in (A, B):
                    for tb in range(4):
                        nc.vector.tensor_copy(
                            vau[h][:, 14 + tb, 0:64], v_sb[tb][:, h * 64:(h + 1) * 64])
                    nc.vector.tensor_copy(vau[h][:, 14:18, 64:65], ones4)
                    expp = expp_p.tile([128, 1024], bf16, name="exppref", tag="exppref")
                    nc.scalar.activation(out=expp, in_=pref[h],
                                         func=mybir.ActivationFunctionType.Exp)
                    exs[h, "pfx"] = expp

                for ck in range(4):
                    for h in (A, B):
                        exs[h, ck] = qk_masks(h, ck)
                        if ck == 1:
                            if h == B:
                                flush_pending()  # previous pair's normalize
                            yps = yaug_p.tile([128, 512], f32, name="yaug", tag="yaug")
                            yp[h] = yps
                            for pb in range(2):
                                nc.tensor.matmul(
                                    yps[0:65, :],
                                    lhsT=vau[h][:, pb, :],
                                    rhs=exs[h, "pfx"][:, pb * 512:(pb + 1) * 512],
                                    start=(pb == 0), stop=False,
                                    skip_group_check=True,
                                )
                            av(yps, h, exs[h, 0], 0, False)
                            del exs[h, 0]
                        elif ck > 1:
                            av(yp[h], h, exs[h, ck - 1], ck - 1, False)
                            del exs[h, ck - 1]
                for h in (A, B):
                    av(yp[h], h, exs[h, 3], 3, True)
                    # 1/denom = exp(-ln(denom)) on ScalarE; broadcast via DMA
                    hrow = (h % 2) * 64
                    mt = h // 2
                    lnd = rcp_p.tile([1, 512], f32, name="lnd", tag="lnd")
                    nc.scalar.activation(out=lnd, in_=yp[h][64:65, :],
                                         func=mybir.ActivationFunctionType.Ln)
                    rcp1 = rcp_p.tile([1, 512], bf16, name="rcp", tag="rcp")
                    nc.scalar.activation(out=rcp1, in_=lnd, scale=-1.0,
                                         func=mybir.ActivationFunctionType.Exp)
                    yunn = rcp_p.tile([64, 512], bf16, name="yunn", tag="yunn")
                    nc.vector.tensor_copy(yunn, yp[h][0:64, :])
                    rb = rcp_p.tile([64, 512], bf16, name="rb", tag="rb")
                    nc.gpsimd.partition_broadcast(out_ap=rb[:, :], in_ap=rcp1[:, :])
                    pending.append((rb, yunn, mt, hrow))
            flush_pending()

        # ---------------- phase 3: output projection (partial) ----------------
        with tc.tile_pool(name="outsb", bufs=3) as out_p, \
             tc.tile_pool(name="cpps", bufs=3, space="PSUM") as cpps_p:
            for tb in range(4):
                for ng in range(2):
                    cps = cpps_p.tile([128, 512], f32, name="cpps", tag="cpps")
                    for ct in range(4):
                        nc.tensor.matmul(
                            cps,
                            lhsT=y_t[ct][:, tb * 128:(tb + 1) * 128],
                            rhs=wp[:, ct, ng * 512:(ng + 1) * 512],
                            start=(ct == 0),
                            stop=(ct == 3),
                        )
                    ob = out_p.tile([128, 512], f32, name="outsb", tag="outsb")
                    nc.scalar.copy(ob, cps)
                    nc.sync.dma_start(
                        out=io["out"].ap()[tb * 128:(tb + 1) * 128, ng * 512:(ng + 1) * 512],
                        in_=ob,
                    )


def build_nc():
    nc = bacc.Bacc("TRN2", target_bir_lowering=False, debug=False)
    io = {}
    io["xT"] = nc.declare_dram_parameter("xT", [128, 8, 512], bf16, isOutput=False)
    for nm in ("wq", "wk", "wv"):
        io[nm] = nc.declare_dram_parameter(nm, [128, 8, 512], bf16, isOutput=False)
    io["constpack"] = nc.declare_dram_parameter("constpack", [128, _CPW], bf16, isOutput=False)
    io["kT_cache"] = nc.declare_dram_parameter("kT_cache", [128, 4, 1536], bf16, isOutput=False)
    io["vpack"] = nc.declare_dram_parameter("vpack", [HPC, 128, 14, 65], bf16, isOutput=False)
    io["prefT"] = nc.declare_dram_parameter("prefT", [HPC, 128, 1024], bf16, isOutput=False)
    io["w_projT"] = nc.declare_dram_parameter("w_projT", [128, 4, 1024], bf16, isOutput=False)
    io["out"] = nc.declare_dram_parameter("out", [512, 1024], f32, isOutput=True)

    with tile_mod.TileContext(nc) as tc:
        _emit(nc, tc, io)
    nc.finalize()
    return nc


def _rotary_tables(start_index):
    half = D // 2
    inv_freq = 1.0 / (ROPE_BASE ** (np.arange(half, dtype=np.float32) / half))
    pos = (float(start_index) + np.arange(T, dtype=np.float32))
    ang = inv_freq[:, None] * pos[None, :]  # (32, 512): [d, t]
    c = np.cos(ang, dtype=np.float32)
    s = np.sin(ang, dtype=np.float32)
    cos2 = np.tile(c, (4, 1))  # (128, 512)
    sin2 = np.tile(np.concatenate([-s, s], axis=0), (2, 1))  # (128, 512)
    perm128 = np.concatenate([_PERM64, 64 + _PERM64])
    return cos2[perm128], sin2[perm128]


def _constpack(start_index):
    cos2, sin2 = _rotary_tables(start_index)
    i = np.arange(128)
    ident = np.eye(128, dtype=np.float32)
    diag = np.where(i[:, None] > i[None, :], MASKVAL, 0.0)
    bound = np.where(i[None, :] > i[:, None], MASKVAL, 0.0)
    cpk = np.empty((128, _CPW), dtype=ml_dtypes.bfloat16)
    cpk[:, _CP_COS:_CP_COS + 512] = cos2
    cpk[:, _CP_SIN:_CP_SIN + 512] = sin2
    cpk[:, _CP_ID:_CP_ID + 128] = ident
    cpk[:, _CP_DG:_CP_DG + 128] = diag
    cpk[:, _CP_CB:_CP_CB + 512] = 0.0
    cpk[:, _CP_CB:_CP_CB + 128] = diag
    cpk[:, _CP_CB + 256:_CP_CB + 384] = bound
    cpk[:, _CP_O4:_CP_O4 + 4] = 1.0
    return np.ascontiguousarray(cpk)


def make_in_maps(x, c_attn_w, c_proj_w, cached_k, cached_v, att_prefix, cache_v, start_index):
    cpk = _constpack(np.asarray(start_index).item())
    qk_perm = np.concatenate([64 * h + _PERM64 for h in range(HPC)])
    bfc = ml_dtypes.bfloat16

    def tile8(mat):  # (1024, 512) -> (128, 8, 512)
        return np.ascontiguousarray(
            mat.reshape(8, 128, 512).transpose(1, 0, 2)).astype(bfc)

    in_maps = []
    for core in range(NCORES):
        b, hg = core // 2, core % 2
        hs = slice(hg * HPC, (hg + 1) * HPC)
        r0, r1 = hg * 512, (hg + 1) * 512
        wq = c_attn_w[r0:r1][qk_perm]
        wk = c_attn_w[C + r0:C + r1][qk_perm]
        wv = c_attn_w[2 * C + r0:2 * C + r1]
        p = att_prefix[b, hs].transpose(0, 2, 1)  # (8, 256, 512)
        prefT = np.ascontiguousarray(
            np.concatenate([p[:, :128], p[:, 128:]], axis=2)).astype(bfc)
        kb = cached_k[b, hs][:, :, _PERM64]  # (8, 1536, 64) perm d
        kT = kb.transpose(0, 2, 1).reshape(4, 128, 1536)  # head-pairs
        kT_cache = np.ascontiguousarray(kT.transpose(1, 0, 2)).astype(bfc)
        vp = np.concatenate([
            cache_v[b, hs].reshape(HPC, 2, 128, D),
            cached_v[b, hs].reshape(HPC, 12, 128, D),
        ], axis=1)  # (8, 14, 128, 64)
        vpack = np.empty((HPC, 128, 14, 65), dtype=bfc)
        vpack[:, :, :, 0:64] = vp.transpose(0, 2, 1, 3).astype(bfc)
        vpack[:, :, :, 64] = 1.0
        wpm = c_proj_w[:, r0:r1].T  # (512, 1024)
        w_projT = np.ascontiguousarray(
            wpm.reshape(4, 128, 1024).transpose(1, 0, 2)).astype(bfc)
        in_maps.append({
            "xT": tile8(x[b].T),
            "wq": tile8(wq.T),
            "wk": tile8(wk.T),
            "wv": tile8(wv.T),
            "constpack": cpk,
            "kT_cache": kT_cache,
            "vpack": np.ascontiguousarray(vpack),
            "prefT": prefT,
            "w_projT": w_projT,
        })
    return in_maps


_NC_CACHE = {}
_DBG = {}


def kernel(x, c_attn_w, c_proj_w, cached_k, cached_v, att_prefix, cache_v, start_index):
    x = np.asarray(x, dtype=np.float32)
    c_attn_w = np.asarray(c_attn_w, dtype=np.float32)
    c_proj_w = np.asarray(c_proj_w, dtype=np.float32)
    cached_k = np.asarray(cached_k, dtype=np.float32)
    cached_v = np.asarray(cached_v, dtype=np.float32)
    att_prefix = np.asarray(att_prefix, dtype=np.float32)
    cache_v = np.asarray(cache_v, dtype=np.float32)

    if "nc" not in _NC_CACHE:
        _NC_CACHE["nc"] = build_nc()
    nc = _NC_CACHE["nc"]

    in_maps = make_in_maps(x, c_attn_w, c_proj_w, cached_k, cached_v,
                           att_prefix, cache_v, start_index)
    from concourse.bass_utils import run_bass_kernel_spmd
    res = run_bass_kernel_spmd(nc, in_maps, list(range(NCORES)))
    outs = res.results
    y = np.empty((B, T, C), dtype=np.float32)
    for b in range(B):
        y[b] = outs[2 * b]["out"] + outs[2 * b + 1]["out"]
    return y


# revision 42
# speedup vs baseline: 1.9424x; 1.2245x over previous
"""Trainium2 Bass kernel for nn_CausalSelfAttention_61795989455492.

Sharding (8 cores): core c -> batch b = c//2, head-group hg = c%2 (8 of 16
heads). Each core runs QKV projection (its head slice), rotary, sliding-window
attention with joint prefix softmax, and a partial output projection over its
512 channel columns. Host sums the two partials per batch (pair reduce).

Device layout ("transposed attention"):
  - q^T, k^T: (d on partition, t on free) straight out of projection matmuls;
    d-rows are PERMUTED per head ([0:16,32:48,16:32,48:64]) so the rotary
    half-swap becomes a within-32-partition DVE stream_shuffle.
  - att^T blocks: (s on partition, t on free); psum slots bank-packed as
    bank0=[slot0(384)|slot3(128)] bank1=[slot1(384)] bank2=[slot2(256)] so
    diag/bound mask matmuls batch two-at-a-time within a bank.
  - softmax denominator via ones-column appended to V (y_aug row 65);
    reciprocal = exp(-ln(denom)) on ScalarE; broadcast across the 64 d-rows
    via a partition-broadcast SBUF->SBUF DMA (no PE involvement).
  - exp via ScalarE with scale=1/sqrt(D), bf16 out; no max subtraction
  - bf16 operands everywhere (PE runs f32r at bf16 speed; halves DMA/DVE)
  - attention processes TWO heads (one kts pair) interleaved to keep the PE
    dense enough that the HAM clock-gate holds 2.4 GHz.
  - input DMA dispatches split across the two HWDGE queues (sync + scalar).
"""

import sys
from contextlib import ExitStack

import numpy as np

sys.path.insert(0, "/opt/trn_rl_repo")

import ml_dtypes  # noqa: E402
import concourse.bass as bass  # noqa: E402
import concourse.tile as tile_mod  # noqa: E402
from concourse import bacc  # noqa: E402
from concourse import mybir  # noqa: E402

B, T, C, H, D = 4, 512, 1024, 16, 64
S_PREV, PFX, WINDOW = 1536, 256, 256
ROPE_BASE = 10000.0
MASKVAL = -1.0e5
HPC = 8  # heads per core
NCORES = 8

f32 = mybir.dt.float32
f32r = mybir.dt.float32r
bf16 = mybir.dt.bfloat16

# window geometry per 512-col KV chunk, transposed layout:
# s-block tj -> t-run [T0[tj], T0[tj]+TN[tj]) at psum offset POFF[tj]
# (bank-packed: no matmul write crosses a 2KB PSUM bank; diag masks sit at
# +0 and bound masks at +256 of their slot so pairs batch within a bank)
# exp packs slots contiguously into ex at offsets EOFF
_T0 = [0, 128, 256, 384]
_TN = [384, 384, 256, 128]
_POFF = [0, 512, 1024, 384]
_EOFF = [0, 512, 896, 384]
_APSW = 1280
_EXPW = 1152

# d-permutation within each head (so rotary swap is within 32 partitions)
_PERM64 = np.concatenate([np.arange(0, 16), np.arange(32, 48),
                          np.arange(16, 32), np.arange(48, 64)])
_SHUF_MASK = list(range(16, 32)) + list(range(0, 16))

# constpack column layout (bf16): cos2 | sin2 | ident | diag | combo | ones4
# combo holds diag at +0 and bound at +256 (gap keeps the [128,2,128] mask AP
# non-contiguous so it isn't flattened and matches the strided psum out)
_CP_COS, _CP_SIN, _CP_ID, _CP_DG, _CP_CB, _CP_O4 = 0, 512, 1024, 1152, 1280, 1792
_CPW = 1796

USE_DMA_BCAST = True  # partition-broadcast SBUF->SBUF DMA for 1/denom


def _emit(nc, tc, io):
    ctx = ExitStack()
    with ctx:
        const = ctx.enter_context(tc.tile_pool(name="const", bufs=1))
        qkrot = ctx.enter_context(tc.tile_pool(name="qkrot", bufs=1))
        vsb = ctx.enter_context(tc.tile_pool(name="vsb", bufs=1))
        ysb = ctx.enter_context(tc.tile_pool(name="ysb", bufs=1))
        vau_p = ctx.enter_context(tc.tile_pool(name="vau", bufs=1))
        pref_p = ctx.enter_context(tc.tile_pool(name="pref", bufs=1))

        # ---- input DMA: phase-1 critical tensors on the sync queue ----
        x_t = const.tile([128, 8, 512], bf16, name="xt", tag="xt")
        nc.sync.dma_start(out=x_t, in_=io["xT"].ap())
        w_t = []
        for s, nm in enumerate(("wq", "wk", "wv")):
            w = const.tile([128, 8, 512], bf16, name=nm, tag=nm)
            nc.sync.dma_start(out=w, in_=io[nm].ap())
            w_t.append(w)
        cp = const.tile([128, _CPW], bf16, name="constpack", tag="constpack")
        nc.sync.dma_start(out=cp, in_=io["constpack"].ap())

        # ---- attention-side tensors, same queue so x/wq get bandwidth first ----
        kts = const.tile([128, 4, 1536], bf16, name="kts", tag="kts")
        nc.sync.dma_start(out=kts, in_=io["kT_cache"].ap())
        vau = []
        pref = []
        for h in range(HPC):
            va = vau_p.tile([128, 18, 65], bf16, name=f"vau{h}", tag=f"vau{h}")
            vau.append(va)
            pf = pref_p.tile([128, 1024], bf16, name=f"pref{h}", tag=f"pref{h}")
            pref.append(pf)
        for h in range(HPC):
            nc.sync.dma_start(out=vau[h][:, 0:14, :], in_=io["vpack"].ap()[h])
            nc.sync.dma_start(out=pref[h], in_=io["prefT"].ap()[h])

        sb_cos = cp[:, _CP_COS:_CP_COS + 512]
        sb_sin = cp[:, _CP_SIN:_CP_SIN + 512]
        sb_I = cp[:, _CP_ID:_CP_ID + 128]
        sb_diag = cp[:, _CP_DG:_CP_DG + 128]
        sb_bound = cp[:, _CP_CB + 256:_CP_CB + 384]
        ones4 = cp[:, _CP_O4:_CP_O4 + 4].rearrange("p (a b) -> p a b", b=1)

        q_rot = [qkrot.tile([128, 512], bf16, name=f"qrot{i}", tag=f"qrot{i}") for i in range(4)]
        k_rot = [qkrot.tile([128, 512], bf16, name=f"krot{i}", tag=f"krot{i}") for i in range(4)]
        v_sb = [vsb.tile([128, 512], bf16, name=f"vsb{i}", tag=f"vsb{i}") for i in range(4)]
        y_t = [ysb.tile([128, 512], bf16, name=f"ysb{i}", tag=f"ysb{i}") for i in range(4)]
        _DBG.update(q_rot=q_rot, k_rot=k_rot, v_sb=v_sb, y_t=y_t, vau=vau)

        # ---------------- phase 1: qkv projection + rotary ----------------
        with tc.tile_pool(name="tmp", bufs=2) as tmp, \
             tc.tile_pool(name="projps", bufs=4, space="PSUM") as projps:
            # (section, mtile); v-items are ('v', tb)
            order = [(0, 0), (1, 0), (0, 1), (1, 1), (2, 0), (0, 2), (1, 2),
                     (2, 1), (0, 3), (1, 3), (2, 2), (2, 3)]
            for sect, m in order:
                ps = projps.tile([128, 512], f32, name="projps", tag="projps")
                if sect < 2:
                    for c in range(8):
                        nc.tensor.matmul(
                            ps,
                            lhsT=w_t[sect][:, c, m * 128:(m + 1) * 128],
                            rhs=x_t[:, c, :],
                            start=(c == 0),
                            stop=(c == 7),
                        )
                    # rotary: rot = qk * cos2 + shuffle(qk) * sin2
                    rot = q_rot[m] if sect == 0 else k_rot[m]
                    qsb = tmp.tile([128, 512], bf16, name="qsb", tag="qsb")
                    nc.vector.tensor_copy(qsb, ps)
                    sh = tmp.tile([128, 512], bf16, name="sh", tag="sh")
                    nc.vector.stream_shuffle(sh, qsb, _SHUF_MASK)
                    nc.vector.tensor_mul(rot, qsb, sb_cos)
                    nc.vector.tensor_mul(sh, sh, sb_sin)
                    nc.vector.tensor_add(rot, rot, sh)
                else:
                    tb = m
                    for c in range(8):
                        nc.tensor.matmul(
                            ps,
                            lhsT=x_t[:, c, tb * 128:(tb + 1) * 128],
                            rhs=w_t[2][:, c, :],
                            start=(c == 0),
                            stop=(c == 7),
                        )
                    nc.vector.tensor_copy(v_sb[tb], ps)

        wp = const.tile([128, 4, 1024], bf16, name="wp", tag="wp")
        nc.sync.dma_start(out=wp, in_=io["w_projT"].ap())

        # ---------------- phase 2: attention, two heads interleaved ----------------
        with tc.tile_pool(name="expsb", bufs=4) as exp_p, \
             tc.tile_pool(name="exppref", bufs=2) as expp_p, \
             tc.tile_pool(name="rcp", bufs=4) as rcp_p, \
             tc.tile_pool(name="attps", bufs=2, space="PSUM") as attps_p, \
             tc.tile_pool(name="yaug", bufs=2, space="PSUM") as yaug_p:
            pending = []

            def qk_masks(h, ck):
                hrow = (h % 2) * 64
                mt = h // 2
                aps = attps_p.tile([128, _APSW], f32, name="attps", tag="attps")

                def qk(tj):
                    t0, tn, off = _T0[tj], _TN[tj], _POFF[tj]
                    if ck < 3:
                        kblk = kts[hrow:hrow + 64, mt,
                                   ck * 512 + tj * 128: ck * 512 + (tj + 1) * 128]
                    else:
                        kblk = k_rot[mt][hrow:hrow + 64, tj * 128:(tj + 1) * 128]
                    nc.tensor.matmul(
                        aps[:, off:off + tn],
                        lhsT=kblk,
                        rhs=q_rot[mt][hrow:hrow + 64, t0:t0 + tn],
                        start=True, stop=False, skip_group_check=True,
                    )

                # bank0: slot0 then its masks, then slot3 (whose start=True
                # re-arms bank0's pending-zero) then its diag
                qk(0)
                nc.tensor.matmul(
                    aps[:, 0:128], lhsT=sb_I, rhs=sb_diag,
                    start=False, stop=False, skip_group_check=True,
                )
                nc.tensor.matmul(
                    aps[:, 256:384], lhsT=sb_I, rhs=sb_bound,
                    start=False, stop=False, skip_group_check=True,
                )
                qk(3)
                nc.tensor.matmul(
                    aps[:, 384:512], lhsT=sb_I, rhs=sb_diag,
                    start=False, stop=False, skip_group_check=True,
                )
                qk(1)
                nc.tensor.matmul(
                    aps[:, 512:640], lhsT=sb_I, rhs=sb_diag,
                    start=False, stop=False, skip_group_check=True,
                )
                nc.tensor.matmul(
                    aps[:, 768:896], lhsT=sb_I, rhs=sb_bound,
                    start=False, stop=False, skip_group_check=True,
                )
                qk(2)
                nc.tensor.matmul(
                    aps[:, 1024:1152], lhsT=sb_I, rhs=sb_diag,
                    start=False, stop=True, skip_group_check=True,
                )
                # exp (scale=1/sqrt(D)), bf16 out, slots packed into ex
                ex = exp_p.tile([128, _EXPW], bf16, name="expsb", tag="expsb")
                nc.scalar.activation(
                    out=ex[:, 0:896], in_=aps[:, 0:896],
                    func=mybir.ActivationFunctionType.Exp, scale=0.125,
                )
                nc.scalar.activation(
                    out=ex[:, 896:1152], in_=aps[:, 1024:1280],
                    func=mybir.ActivationFunctionType.Exp, scale=0.125,
                )
                return ex

            def av(yps, h, ex, ck, last):
                for tj in range(4):
                    t0, tn, eoff = _T0[tj], _TN[tj], _EOFF[tj]
                    blk = (2 + ck * 4 + tj) if ck < 3 else (14 + tj)
                    nc.tensor.matmul(
                        yps[0:65, t0:t0 + tn],
                        lhsT=vau[h][:, blk, :],
                        rhs=ex[:, eoff:eoff + tn],
                        start=False,
                        stop=(last and tj == 3),
                        skip_group_check=True,
                    )

            def flush_pending():
                while pending:
                    rb_o, yunn_o, mt_o, hrow_o = pending.pop(0)
                    nc.vector.tensor_mul(
                        y_t[mt_o][hrow_o:hrow_o + 64, :], yunn_o, rb_o)

            for p in range(4):
                A, B = 2 * p, 2 * p + 1
                exs = {}
                yp = {}
                for h in (A, B):
                    for tb in range(4):
                        nc.vector.tensor_copy(
                            vau[h][:, 14 + tb, 0:64], v_sb[tb][:, h * 64:(h + 1) * 64])
                    nc.vector.tensor_copy(vau[h][:, 14:18, 64:65], ones4)
                    expp = expp_p.tile([128, 1024], bf16, name="exppref", tag="exppref")
                    nc.scalar.activation(out=expp, in_=pref[h],
                                         func=mybir.ActivationFunctionType.Exp)
                    exs[h, "pfx"] = expp

                for ck in range(4):
                    for h in (A, B):
                        exs[h, ck] = qk_masks(h, ck)
                        if ck == 1:
                            if h == B:
                                flush_pending()  # previous pair's normalize
                            yps = yaug_p.tile([128, 512], f32, name="yaug", tag="yaug")
                            yp[h] = yps
                            for pb in range(2):
                                nc.tensor.matmul(
                                    yps[0:65, :],
                                    lhsT=vau[h][:, pb, :],
                                    rhs=exs[h, "pfx"][:, pb * 512:(pb + 1) * 512],
                                    start=(pb == 0), stop=False,
                                    skip_group_check=True,
                                )
                            av(yps, h, exs[h, 0], 0, False)
                            del exs[h, 0]
                        elif ck > 1:
                            av(yp[h], h, exs[h, ck - 1], ck - 1, False)
                            del exs[h, ck - 1]
                for h in (A, B):
                    av(yp[h], h, exs[h, 3], 3, True)
                    # 1/denom = exp(-ln(denom)) on ScalarE; broadcast via DMA
                    hrow = (h % 2) * 64
                    mt = h // 2
                    den = rcp_p.tile([1, 512], f32, name="den", tag="den")
                    nc.vector.tensor_copy(den, yp[h][64:65, :])
                    rcp1 = rcp_p.tile([1, 512], f32, name="rcp", tag="rcp")
                    nc.vector.reciprocal_approx_fast(out=rcp1, in_=den)
                    yunn = rcp_p.tile([64, 512], bf16, name="yunn", tag="yunn")
                    nc.vector.tensor_copy(yunn, yp[h][0:64, :])
                    rb = rcp_p.tile([64, 512], f32, name="rb", tag="rb")
                    nc.gpsimd.partition_broadcast(out_ap=rb[:, :], in_ap=rcp1[:, :])
                    pending.append((rb, yunn, mt, hrow))
            flush_pending()

        # ---------------- phase 3: output projection (partial) ----------------
        with tc.tile_pool(name="outsb", bufs=3) as out_p, \
             tc.tile_pool(name="cpps", bufs=3, space="PSUM") as cpps_p:
            for tb in range(4):
                for ng in range(2):
                    cps = cpps_p.tile([128, 512], f32, name="cpps", tag="cpps")
                    for ct in range(4):
                        nc.tensor.matmul(
                            cps,
                            lhsT=y_t[ct][:, tb * 128:(tb + 1) * 128],
                            rhs=wp[:, ct, ng * 512:(ng + 1) * 512],
                            start=(ct == 0),
                            stop=(ct == 3),
                        )
                    ob = out_p.tile([128, 512], f32, name="outsb", tag="outsb")
                    nc.scalar.copy(ob, cps)
                    nc.sync.dma_start(
                        out=io["out"].ap()[tb * 128:(tb + 1) * 128, ng * 512:(ng + 1) * 512],
                        in_=ob,
                    )


def build_nc():
    nc = bacc.Bacc("TRN2", target_bir_lowering=False, debug=False)
    io = {}
    io["xT"] = nc.declare_dram_parameter("xT", [128, 8, 512], bf16, isOutput=False)
    for nm in ("wq", "wk", "wv"):
        io[nm] = nc.declare_dram_parameter(nm, [128, 8, 512], bf16, isOutput=False)
    io["constpack"] = nc.declare_dram_parameter("constpack", [128, _CPW], bf16, isOutput=False)
    io["kT_cache"] = nc.declare_dram_parameter("kT_cache", [128, 4, 1536], bf16, isOutput=False)
    io["vpack"] = nc.declare_dram_parameter("vpack", [HPC, 128, 14, 65], bf16, isOutput=False)
    io["prefT"] = nc.declare_dram_parameter("prefT", [HPC, 128, 1024], bf16, isOutput=False)
    io["w_projT"] = nc.declare_dram_parameter("w_projT", [128, 4, 1024], bf16, isOutput=False)
    io["out"] = nc.declare_dram_parameter("out", [512, 1024], f32, isOutput=True)

    with tile_mod.TileContext(nc) as tc:
        _emit(nc, tc, io)
    nc.finalize()
    return nc


def _rotary_tables(start_index):
    half = D // 2
    inv_freq = 1.0 / (ROPE_BASE ** (np.arange(half, dtype=np.float32) / half))
    pos = (float(start_index) + np.arange(T, dtype=np.float32))
    ang = inv_freq[:, None] * pos[None, :]  # (32, 512): [d, t]
    c = np.cos(ang, dtype=np.float32)
    s = np.sin(ang, dtype=np.float32)
    cos2 = np.tile(c, (4, 1))  # (128, 512)
    sin2 = np.tile(np.concatenate([-s, s], axis=0), (2, 1))  # (128, 512)
    perm128 = np.concatenate([_PERM64, 64 + _PERM64])
    return cos2[perm128], sin2[perm128]


def _constpack(start_index):
    cos2, sin2 = _rotary_tables(start_index)
    i = np.arange(128)
    ident = np.eye(128, dtype=np.float32)
    diag = np.where(i[:, None] > i[None, :], MASKVAL, 0.0)
    bound = np.where(i[None, :] > i[:, None], MASKVAL, 0.0)
    cpk = np.empty((128, _CPW), dtype=ml_dtypes.bfloat16)
    cpk[:, _CP_COS:_CP_COS + 512] = cos2
    cpk[:, _CP_SIN:_CP_SIN + 512] = sin2
    cpk[:, _CP_ID:_CP_ID + 128] = ident
    cpk[:, _CP_DG:_CP_DG + 128] = diag
    cpk[:, _CP_CB:_CP_CB + 512] = 0.0
    cpk[:, _CP_CB:_CP_CB + 128] = diag
    cpk[:, _CP_CB + 256:_CP_CB + 384] = bound
    cpk[:, _CP_O4:_CP_O4 + 4] = 1.0
    return np.ascontiguousarray(cpk)


def make_in_maps(x, c_attn_w, c_proj_w, cached_k, cached_v, att_prefix, cache_v, start_index):
    cpk = _constpack(np.asarray(start_index).item())
    qk_perm = np.concatenate([64 * h + _PERM64 for h in range(HPC)])
    bfc = ml_dtypes.bfloat16

    def tile8(mat):  # (1024, 512) -> (128, 8, 512)
        return np.ascontiguousarray(
            mat.reshape(8, 128, 512).transpose(1, 0, 2)).astype(bfc)

    in_maps = []
    for core in range(NCORES):
        b, hg = core // 2, core % 2
        hs = slice(hg * HPC, (hg + 1) * HPC)
        r0, r1 = hg * 512, (hg + 1) * 512
        wq = c_attn_w[r0:r1][qk_perm]
        wk = c_attn_w[C + r0:C + r1][qk_perm]
        wv = c_attn_w[2 * C + r0:2 * C + r1]
        p = att_prefix[b, hs].transpose(0, 2, 1)  # (8, 256, 512)
        prefT = np.ascontiguousarray(
            np.concatenate([p[:, :128], p[:, 128:]], axis=2)).astype(bfc)
        kb = cached_k[b, hs][:, :, _PERM64]  # (8, 1536, 64) perm d
        kT = kb.transpose(0, 2, 1).reshape(4, 128, 1536)  # head-pairs
        kT_cache = np.ascontiguousarray(kT.transpose(1, 0, 2)).astype(bfc)
        vp = np.concatenate([
            cache_v[b, hs].reshape(HPC, 2, 128, D),
            cached_v[b, hs].reshape(HPC, 12, 128, D),
        ], axis=1)  # (8, 14, 128, 64)
        vpack = np.empty((HPC, 128, 14, 65), dtype=bfc)
        vpack[:, :, :, 0:64] = vp.transpose(0, 2, 1, 3).astype(bfc)
        vpack[:, :, :, 64] = 1.0
        wpm = c_proj_w[:, r0:r1].T  # (512, 1024)
        w_projT = np.ascontiguousarray(
            wpm.reshape(4, 128, 1024).transpose(1, 0, 2)).astype(bfc)
        in_maps.append({
            "xT": tile8(x[b].T),
            "wq": tile8(wq.T),
            "wk": tile8(wk.T),
            "wv": tile8(wv.T),
            "constpack": cpk,
            "kT_cache": kT_cache,
            "vpack": np.ascontiguousarray(vpack),
            "prefT": prefT,
            "w_projT": w_projT,
        })
    return in_maps


_NC_CACHE = {}
_DBG = {}


def kernel(x, c_attn_w, c_proj_w, cached_k, cached_v, att_prefix, cache_v, start_index):
    x = np.asarray(x, dtype=np.float32)
    c_attn_w = np.asarray(c_attn_w, dtype=np.float32)
    c_proj_w = np.asarray(c_proj_w, dtype=np.float32)
    cached_k = np.asarray(cached_k, dtype=np.float32)
    cached_v = np.asarray(cached_v, dtype=np.float32)
    att_prefix = np.asarray(att_prefix, dtype=np.float32)
    cache_v = np.asarray(cache_v, dtype=np.float32)

    if "nc" not in _NC_CACHE:
        _NC_CACHE["nc"] = build_nc()
    nc = _NC_CACHE["nc"]

    in_maps = make_in_maps(x, c_attn_w, c_proj_w, cached_k, cached_v,
                           att_prefix, cache_v, start_index)
    from concourse.bass_utils import run_bass_kernel_spmd
    res = run_bass_kernel_spmd(nc, in_maps, list(range(NCORES)))
    outs = res.results
    y = np.empty((B, T, C), dtype=np.float32)
    for b in range(B):
        y[b] = outs[2 * b]["out"] + outs[2 * b + 1]["out"]
    return y
